# revision 25
# baseline (speedup 1.0000x reference)
"""AdaLN (DiT-style) transformer block on 8 Trainium2 NeuronCores.

Data-parallel over batch: core b computes batch element b end-to-end
(B == n_cores == 8), so no collectives are needed. All large matmuls run
in float32r (fp32 storage, full PE rate for moving dim >= 256).

Layout strategy per core:
  - LN in natural [i, d]; h transposed on PE; AdaLN modulate folded into the
    PSUM->SBUF transpose copies as per-partition scale/bias (columns).
  - q,k produced transposed [d_head, i]; scores computed transposed
    S^T[j, i] = k . q so softmax-exp output feeds the PV matmul directly as
    the moving operand; denominators via a ones column appended to V
    (M=65 PV matmuls); no max subtraction (|scores*scale| ~ 8.5).
  - MLP hidden produced transposed so mlp2 contracts over partitions.

Fixed problem shape: x [8, 1024, 384], cond [8, 384], H=6 heads, hd=64.
"""
import sys

if '/opt/trn_rl_repo' not in sys.path:
    sys.path.insert(0, '/opt/trn_rl_repo')

import ml_dtypes
import numpy as np

import concourse.bacc as bacc
import concourse.tile as tile
from concourse import masks, mybir
from concourse.bass_utils import run_bass_kernel_spmd

B, L, D, H = 8, 1024, 384, 6
HD = D // H                  # 64
DQ = 3 * D                   # 1152
DM = 4 * D                   # 1536
DC = 6 * D                   # 2304
KD = D // 128                # 3 k-tiles over D
IT = L // 128                # 8 i-tiles over L
IC = L // 512                # 2 512-chunks over L
SCALE = HD ** -0.5
EPS = 1e-5

f32 = mybir.dt.float32
f32r = mybir.dt.float32r
bf16 = mybir.dt.bfloat16
ACTF = mybir.ActivationFunctionType
ALU = mybir.AluOpType

_cache = {}
C_ORDER = (0, 1, 3, 4, 2, 5)   # s1 b1 s2 b2 g1 g2: branch-1 modulate first


def _layernorm(nc, sb, xt, eps_t, out):
    """out = LN(xt) over the free dim ([128, D] tiles), no affine."""
    stats = sb.tile([128, 6], f32, name="ln_stats", tag="ln_stats")
    nc.vector.bn_stats(out=stats, in_=xt)
    mv = sb.tile([128, 2], f32, name="ln_mv", tag="ln_mv")
    nc.vector.bn_aggr(out=mv, in_=stats)
    rstd = sb.tile([128, 1], f32, name="ln_rstd", tag="ln_rstd")
    nc.scalar.activation(out=rstd, in_=mv[:, 1:2], func=ACTF.Sqrt, bias=eps_t, scale=1.0)
    nc.vector.reciprocal_approx_fast(rstd, rstd)
    negmr = sb.tile([128, 1], f32, name="ln_negmr", tag="ln_negmr")
    nc.vector.tensor_scalar(out=negmr, in0=mv[:, 0:1], scalar1=rstd, scalar2=-1.0,
                            op0=ALU.mult, op1=ALU.mult)
    nc.vector.tensor_scalar(out=out, in0=xt, scalar1=rstd, scalar2=negmr,
                            op0=ALU.mult, op1=ALU.add)


def build(flags):
    """Build the per-core Bass program. flags: (cond_b, qkv_b, proj_b, m1b, m2b)."""
    use_cb, use_qb, use_pb, use_m1, use_m2 = flags
    nc = bacc.Bacc()

    xb = nc.declare_dram_parameter("xb", [L, D], f32, isOutput=False)
    cond = nc.declare_dram_parameter("cond", [D], f32, isOutput=False)
    cond_wT = nc.declare_dram_parameter("cond_wT", [D, DC], f32r, isOutput=False)
    qkv_wT = nc.declare_dram_parameter("qkv_wT", [D, DQ], f32r, isOutput=False)
    proj_wT = nc.declare_dram_parameter("proj_wT", [D, D], f32r, isOutput=False)
    w1T = nc.declare_dram_parameter("w1T", [D, DM], f32r, isOutput=False)
    w2T = nc.declare_dram_parameter("w2T", [DM, D], f32r, isOutput=False)
    if use_cb:
        cond_b = nc.declare_dram_parameter("cond_b", [DC], f32, isOutput=False)
    if use_qb:
        qkv_b = nc.declare_dram_parameter("qkv_b", [DQ], f32, isOutput=False)
    if use_pb:
        proj_b = nc.declare_dram_parameter("proj_b", [D], f32r, isOutput=False)
    if use_m1:
        mlp_b1 = nc.declare_dram_parameter("mlp_b1", [DM], f32, isOutput=False)
    if use_m2:
        mlp_b2 = nc.declare_dram_parameter("mlp_b2", [D], f32r, isOutput=False)
    out = nc.declare_dram_parameter("out", [L, D], f32, isOutput=True)

    with tile.TileContext(nc) as tc:
        from contextlib import ExitStack
        ctx = ExitStack()
        with ctx:
            persist = ctx.enter_context(tc.tile_pool(name="persist", bufs=1))
            sb = ctx.enter_context(tc.tile_pool(name="small", bufs=4))
            hpool = ctx.enter_context(tc.tile_pool(name="hpool", bufs=3))
            exps = ctx.enter_context(tc.tile_pool(name="exps", bufs=3))
            dramp = ctx.enter_context(tc.tile_pool(name="dramp", bufs=1, space="DRAM"))
            # PSUM budget (8 banks): ps_w 2 + ps_x 2 + ps_s 2x[128,1024] = 8
            ps_w = ctx.enter_context(tc.tile_pool(name="ps_w", bufs=2, space="PSUM"))
            ps_x = ctx.enter_context(tc.tile_pool(name="ps_x", bufs=2, space="PSUM"))

            # ---------------- DMAs on the critical path first ----------------
            cvec = persist.tile([128, KD], f32, tag="cvec")
            nc.sync.dma_start(out=cvec, in_=cond[:].rearrange("(k p) -> p k", p=128))
            xt = [persist.tile([128, D], f32, name=f"x{i}", tag=f"x{i}") for i in range(IT)]
            for i in range(2):
                nc.sync.dma_start(out=xt[i], in_=xb[i * 128:(i + 1) * 128, :])
            cwp = tc.tile_pool(name="condw", bufs=1)
            cwpool = cwp.__enter__()
            condw_sb = [cwpool.tile([128, DC], f32r, name=f"condw{k}", tag=f"condw{k}")
                        for k in range(KD)]
            for c in C_ORDER:
                for k in range(KD):
                    nc.sync.dma_start(out=condw_sb[k][:, c * D:(c + 1) * D],
                                      in_=cond_wT[k * 128:(k + 1) * 128, c * D:(c + 1) * D])

            for i in range(2, IT):
                nc.sync.dma_start(out=xt[i], in_=xb[i * 128:(i + 1) * 128, :])
            qkvw_sb = [persist.tile([128, DQ], f32r, name=f"qkvw{k}", tag=f"qkvw{k}")
                       for k in range(KD)]
            for k in range(KD):
                nc.sync.dma_start(out=qkvw_sb[k], in_=qkv_wT[k * 128:(k + 1) * 128, :])
            projw_sb = [persist.tile([128, D], f32r, name=f"projw{k}", tag=f"projw{k}")
                        for k in range(KD)]
            w1_sb = [persist.tile([128, DM], f32r, name=f"w1_{k}", tag=f"w1_{k}")
                     for k in range(KD)]
            w2_sb = [persist.tile([128, D], f32r, name=f"w2_{k}", tag=f"w2_{k}")
                     for k in range(12)]

            # ---------------- constants ----------------
            ident = persist.tile([128, 128], f32, tag="ident")
            masks.make_identity(nc, ident[:, :])
            # PE warmup: ~7us of dummy matmul keeps the HAM clock-gate open
            # (K=8/8, 2.4 GHz) by the time the first real matmuls arrive.
            warm_ps = ps_w.tile([128, 128], f32, name="warm_ps", tag="mm")
            for w in range(16):
                nc.tensor.matmul(warm_ps[:, :], ident[:, :], ident[:, :],
                                 start=(w == 0), stop=(w == 15))
            wsink = sb.tile([1, 1], f32, name="wsink", tag="wsink")
            nc.scalar.copy(wsink, warm_ps[0:1, 0:1])
            wdram = dramp.tile([1, 1], f32, name="wdram", tag="wdram")
            nc.sync.dma_start(out=wdram, in_=wsink)

            def pe_filler(n, pool, tg):
                wp = pool.tile([128, 128], f32, name="warm2", tag=tg)
                for w in range(n):
                    nc.tensor.matmul(wp[:, :], ident[:, :], ident[:, :],
                                     start=(w == 0), stop=(w == n - 1))
                ws = sb.tile([1, 1], f32, name="wsink", tag="wsink")
                nc.scalar.copy(ws, wp[0:1, 0:1])
                nc.sync.dma_start(out=wdram, in_=ws)
            eps_t = persist.tile([128, 1], f32, tag="eps")
            nc.vector.memset(eps_t, EPS)
            ones6_f = persist.tile([128, 6, 1], f32, tag="ones6f")
            nc.vector.memset(ones6_f, 1.0)

            if use_qb:
                qkcols = persist.tile([128, 6], f32, tag="qkcols")
                nc.sync.dma_start(out=qkcols, in_=qkv_b[0:768].rearrange("(t p) -> p t", p=128))
                vbrow = persist.tile([1, D], f32, tag="vbrow")
                nc.sync.dma_start(out=vbrow, in_=qkv_b[768:1152].rearrange("(o f) -> o f", o=1))
                vb_bc = persist.tile([128, D], f32, tag="vb_bc")
                nc.gpsimd.partition_broadcast(vb_bc, vbrow[:1, :])
            if use_m1:
                m1cols = persist.tile([128, 12], f32, tag="m1cols")
                nc.sync.dma_start(out=m1cols, in_=mlp_b1[:].rearrange("(t p) -> p t", p=128))
            onesr = None
            if use_pb or use_m2:
                ones_f = persist.tile([1, 128], f32, tag="onesf")
                nc.vector.memset(ones_f, 1.0)
                onesr = persist.tile([1, 128], f32r, tag="onesr")
                nc.vector.tensor_copy(onesr, ones_f)
            if use_pb:
                pbrow = persist.tile([1, D], f32r, tag="pbrow")
                nc.sync.dma_start(out=pbrow, in_=proj_b[:].rearrange("(o f) -> o f", o=1))
            if use_m2:
                m2row = persist.tile([1, D], f32r, tag="m2row")
                nc.sync.dma_start(out=m2row, in_=mlp_b2[:].rearrange("(o f) -> o f", o=1))

            # ---------------- conditioning ----------------
            # c = SiLU(cond) @ cond_wT (+ cond_b); chunks: s1 b1 g1 s2 b2 g2.
            # s/b chunks go to DRAM and come back as [128, KD] columns so the
            # modulate can fold into transpose copies as per-partition affine.
            # (1 + s) is applied before the round trip. g chunks broadcast.
            scond = persist.tile([128, KD], f32r, tag="scond")
            nc.scalar.activation(out=scond, in_=cvec, func=ACTF.Silu)
            g1bc = persist.tile([128, D], f32, tag="g1bc")
            g2bc = persist.tile([128, D], f32, tag="g2bc")
            s1c = persist.tile([128, KD], f32, tag="s1c")
            b1c = persist.tile([128, KD], f32, tag="b1c")
            s2c = persist.tile([128, KD], f32, tag="s2c")
            b2c = persist.tile([128, KD], f32, tag="b2c")
            col_dst = {0: s1c, 1: b1c, 3: s2c, 4: b2c}
            for c in C_ORDER:
                pc = ps_w.tile([1, D], f32, name="pc", tag="mm")
                for k in range(KD):
                    nc.tensor.matmul(pc[:, :], scond[:, k:k + 1],
                                     condw_sb[k][:, c * D:(c + 1) * D],
                                     start=(k == 0), stop=(k == KD - 1))
                cseg = sb.tile([1, D], f32, name="cseg", tag="cseg")
                nc.scalar.copy(cseg, pc[:, :])
                if use_cb:
                    cbseg = sb.tile([1, D], f32, name="cbseg", tag="cbseg")
                    nc.sync.dma_start(out=cbseg,
                                      in_=cond_b[c * D:(c + 1) * D].rearrange("(o f) -> o f", o=1))
                    nc.vector.tensor_add(cseg, cseg, cbseg)
                if c in (0, 3):
                    nc.scalar.add(cseg, cseg, 1.0)   # 1 + scale
                if c in col_dst:
                    # row -> columns via tiny PE transposes ([1,128] -> [128,1])
                    for k in range(KD):
                        ptc = ps_x.tile([128, 1], f32, name="ptc", tag="tp")
                        nc.tensor.transpose(ptc[:, :], cseg[0:1, k * 128:(k + 1) * 128],
                                            ident[0:1, 0:1])
                        nc.vector.tensor_copy(col_dst[c][:, k:k + 1], ptc[:, :])
                else:
                    nc.gpsimd.partition_broadcast(g1bc if c == 2 else g2bc, cseg[:1, :])
            cwp.__exit__(None, None, None)

            def modcopy(dst_ap, src_ap, sc, bc, k, use_act):
                if use_act:
                    nc.scalar.activation(out=dst_ap, in_=src_ap, func=ACTF.Identity,
                                         scale=sc[:, k:k + 1], bias=bc[:, k:k + 1])
                else:
                    nc.vector.tensor_scalar(out=dst_ap, in0=src_ap,
                                            scalar1=sc[:, k:k + 1], scalar2=bc[:, k:k + 1],
                                            op0=ALU.mult, op1=ALU.add)

            # attnT outlives the attention-phase scratch (consumed by proj)
            with tc.tile_pool(name="attn2", bufs=1) as ap2:
                attnT = [ap2.tile([128, L], f32r, name=f"attnT{k}", tag=f"attnT{k}")
                         for k in range(KD)]

                with tc.tile_pool(name="attn1", bufs=1) as ap1:
                    lnT = [ap1.tile([128, L], f32r, name=f"lnT{k}", tag=f"lnT{k}")
                           for k in range(KD)]
                    qkT = [ap1.tile([128, L], f32r, name=f"qkT{t}", tag=f"qkT{t}")
                           for t in range(6)]
                    vsb = [ap1.tile([128, 6 * (HD + 1)], f32r, name=f"v{j}", tag=f"v{j}")
                           for j in range(IT)]

                    # LN1 -> transpose -> modulate-copy -> lnT; then v(i)
                    for i in range(IT):
                        ln = hpool.tile([128, D], f32, name="ln1", tag="h1")
                        _layernorm(nc, sb, xt[i], eps_t, ln)
                        for k in range(KD):
                            pt = ps_x.tile([128, 128], f32, name="pt", tag="tp")
                            nc.tensor.transpose(pt[:, :], ln[:, k * 128:(k + 1) * 128],
                                                ident[:, :])
                            modcopy(lnT[k][:, i * 128:(i + 1) * 128], pt[:, :],
                                    s1c, b1c, k, (i + k) % 2 == 0)
                        # v for this i-tile (natural layout + ones column)
                        pv = ps_w.tile([128, D], f32, name="pv", tag="mm")
                        for k in range(KD):
                            nc.tensor.matmul(pv[:, :],
                                             lnT[k][:, i * 128:(i + 1) * 128],
                                             qkvw_sb[k][:, 768:1152],
                                             start=(k == 0), stop=(k == KD - 1))
                        vview = vsb[i][:, :].rearrange("p (h c) -> p h c", c=HD + 1)
                        pvview = pv[:, :].rearrange("p (h c) -> p h c", c=HD)
                        if use_qb:
                            nc.vector.tensor_add(vview[:, :, 0:HD], pvview,
                                                 vb_bc[:, :].rearrange("p (h c) -> p h c", c=HD))
                        else:
                            nc.vector.tensor_copy(vview[:, :, 0:HD], pvview)
                        nc.gpsimd.tensor_copy(vview[:, :, HD:HD + 1], ones6_f)

                    def emit_qkT(t):
                        for ic in range(IC):
                            pq = ps_w.tile([128, 512], f32, name="pq", tag="mm")
                            for k in range(KD):
                                nc.tensor.matmul(pq[:, :],
                                                 qkvw_sb[k][:, t * 128:(t + 1) * 128],
                                                 lnT[k][:, ic * 512:(ic + 1) * 512],
                                                 start=(k == 0), stop=(k == KD - 1))
                            if use_qb:
                                nc.scalar.activation(out=qkT[t][:, ic * 512:(ic + 1) * 512],
                                                     in_=pq[:, :], func=ACTF.Copy,
                                                     bias=qkcols[:, t:t + 1], scale=1.0)
                            else:
                                nc.vector.tensor_copy(qkT[t][:, ic * 512:(ic + 1) * 512],
                                                      pq[:, :])

                    # attention; S^T[j, i] per (head, j-tile), exp over full i,
                    # software-pipelined: S/exp one j-tile ahead of PV.
                    # qkT for head pair p is emitted just before its heads.
                    for t in (0, 3, 1, 4, 2, 5):
                        emit_qkT(t)
                    ps_s_cm = tc.tile_pool(name="ps_s", bufs=2, space="PSUM")
                    ps_s = ps_s_cm.__enter__()
                    for h in range(H):
                        tq = h // 2
                        ro = (h % 2) * HD
                        po_pool = ps_x if h % 2 == 0 else ps_w
                        po = [po_pool.tile([HD + 1, 512], f32, name=f"po{ic}",
                                           tag="tp" if h % 2 == 0 else "mm")
                              for ic in range(IC)]
                        es_t = [None] * IT

                        def emit_s(jt):
                            pss = ps_s.tile([128, L], f32, name="pss", tag="s")
                            for ic in range(IC):
                                nc.tensor.matmul(pss[:, ic * 512:(ic + 1) * 512],
                                                 qkT[3 + tq][ro:ro + HD, jt * 128:(jt + 1) * 128],
                                                 qkT[tq][ro:ro + HD, ic * 512:(ic + 1) * 512],
                                                 start=True, stop=True)
                            es = exps.tile([128, L], f32r, name="es", tag="expS")
                            nc.scalar.activation(out=es, in_=pss[:, :], func=ACTF.Exp,
                                                 scale=SCALE)
                            es_t[jt] = es

                        def emit_pv(jt):
                            for ic in range(IC):
                                nc.tensor.matmul(po[ic][:, :],
                                                 vsb[jt][:, h * (HD + 1):(h + 1) * (HD + 1)],
                                                 es_t[jt][:, ic * 512:(ic + 1) * 512],
                                                 start=(jt == 0), stop=(jt == IT - 1))

                        emit_s(0)
                        for jt in range(1, IT):
                            emit_s(jt)
                            emit_pv(jt - 1)
                        emit_pv(IT - 1)

                        for ic in range(IC):
                            dn = sb.tile([1, 512], f32, name="dn", tag="dn", bufs=3)
                            nc.vector.tensor_copy(dn, po[ic][HD:HD + 1, :])
                            rcp = sb.tile([1, 512], f32, name="rcp", tag="rcp", bufs=3)
                            nc.vector.reciprocal_approx_fast(rcp, dn)
                            rcpb = sb.tile([HD, 512], f32, name="rcpb", tag="rcpb", bufs=3)
                            nc.gpsimd.partition_broadcast(rcpb, rcp[:1, :])
                            nc.vector.tensor_mul(
                                attnT[tq][ro:ro + HD, ic * 512:(ic + 1) * 512],
                                po[ic][0:HD, :], rcpb)

                ps_s_cm.__exit__(None, None, None)
                # ---------------- proj + residual ----------------
                x1t = [persist.tile([128, D], f32, name=f"x1_{i}", tag=f"x1_{i}")
                       for i in range(IT)]
                for k in range(KD):
                    nc.sync.dma_start(out=projw_sb[k], in_=proj_wT[k * 128:(k + 1) * 128, :])
                for k in range(KD):
                    nc.sync.dma_start(out=w1_sb[k], in_=w1T[k * 128:(k + 1) * 128, :])
                for k in range(12):
                    nc.sync.dma_start(out=w2_sb[k], in_=w2T[k * 128:(k + 1) * 128, :])
                for i in range(IT):
                    py = ps_w.tile([128, D], f32, name="py", tag="mm")
                    for k in range(KD):
                        nc.tensor.matmul(py[:, :],
                                         attnT[k][:, i * 128:(i + 1) * 128],
                                         projw_sb[k][:, :],
                                         start=(k == 0), stop=(k == KD - 1 and not use_pb))
                    if use_pb:
                        nc.tensor.matmul(py[:, :], onesr[:1, :], pbrow[:1, :],
                                         start=False, stop=True)
                    t1 = hpool.tile([128, D], f32, name="t1", tag="t1")
                    nc.vector.tensor_mul(t1, py[:, :], g1bc)
                    nc.vector.tensor_add(x1t[i], xt[i], t1)

            # ---------------- LN2 + MLP ----------------
            with tc.tile_pool(name="mlp1", bufs=1) as mp1, \
                 tc.tile_pool(name="ps_m", bufs=4, space="PSUM") as ps_m:
                h2T = [mp1.tile([128, L], f32r, name=f"h2T{k}", tag=f"h2T{k}")
                       for k in range(KD)]
                siluT = [mp1.tile([128, L], f32r, name=f"siluT{t}", tag=f"siluT{t}")
                         for t in range(12)]

                pe_filler(12, ps_m, "m")
                for i in range(IT):
                    ln = hpool.tile([128, D], f32, name="ln2", tag="h2")
                    _layernorm(nc, sb, x1t[i], eps_t, ln)
                    for k in range(KD):
                        pt = ps_x.tile([128, 128], f32, name="pt2", tag="tp")
                        nc.tensor.transpose(pt[:, :], ln[:, k * 128:(k + 1) * 128],
                                            ident[:, :])
                        modcopy(h2T[k][:, i * 128:(i + 1) * 128], pt[:, :],
                                s2c, b2c, k, (i + k) % 2 == 0)

                # mlp1 (transposed out) + SiLU; ic outer so mlp2 can start
                # on the first half while the second half still runs
                for ic in range(IC):
                    for t in range(12):
                        pa = ps_m.tile([128, 512], f32, name="pa", tag="m")
                        for k in range(KD):
                            nc.tensor.matmul(pa[:, :],
                                             w1_sb[k][:, t * 128:(t + 1) * 128],
                                             h2T[k][:, ic * 512:(ic + 1) * 512],
                                             start=(k == 0), stop=(k == KD - 1))
                        if use_m1:
                            nc.scalar.activation(out=siluT[t][:, ic * 512:(ic + 1) * 512],
                                                 in_=pa[:, :], func=ACTF.Silu,
                                                 bias=m1cols[:, t:t + 1], scale=1.0)
                        else:
                            nc.scalar.activation(out=siluT[t][:, ic * 512:(ic + 1) * 512],
                                                 in_=pa[:, :], func=ACTF.Silu)

                # mlp2 + residual + store
                for i in range(IT):
                    py = ps_m.tile([128, D], f32, name="py2", tag="m")
                    for k in range(12):
                        nc.tensor.matmul(py[:, :],
                                         siluT[k][:, i * 128:(i + 1) * 128],
                                         w2_sb[k][:, :],
                                         start=(k == 0), stop=(k == 11 and not use_m2))
                    if use_m2:
                        nc.tensor.matmul(py[:, :], onesr[:1, :], m2row[:1, :],
                                         start=False, stop=True)
                    t2 = hpool.tile([128, D], f32, name="t2", tag="t2")
                    nc.vector.tensor_mul(t2, py[:, :], g2bc)
                    ot = hpool.tile([128, D], f32, name="ot", tag="ot")
                    nc.gpsimd.tensor_add(ot, x1t[i], t2)
                    nc.sync.dma_start(out=out[i * 128:(i + 1) * 128, :], in_=ot)

    nc.compile()
    return nc


def _get_nc(flags):
    if flags not in _cache:
        _cache[flags] = build(flags)
    return _cache[flags]


def kernel(x, cond, qkv_w, qkv_b, proj_w, proj_b, mlp_w1, mlp_b1, mlp_w2, mlp_b2,
           cond_w, cond_b, num_heads):
    x = np.asarray(x, np.float32)
    cond = np.asarray(cond, np.float32)
    qkv_w = np.asarray(qkv_w, np.float32)
    qkv_b = np.asarray(qkv_b, np.float32)
    proj_w = np.asarray(proj_w, np.float32)
    proj_b = np.asarray(proj_b, np.float32)
    mlp_w1 = np.asarray(mlp_w1, np.float32)
    mlp_b1 = np.asarray(mlp_b1, np.float32)
    mlp_w2 = np.asarray(mlp_w2, np.float32)
    mlp_b2 = np.asarray(mlp_b2, np.float32)
    cond_w = np.asarray(cond_w, np.float32)
    cond_b = np.asarray(cond_b, np.float32)
    assert int(num_heads) == H and x.shape == (B, L, D)

    flags = (bool(cond_b.any()), bool(qkv_b.any()), bool(proj_b.any()),
             bool(mlp_b1.any()), bool(mlp_b2.any()))
    nc = _get_nc(flags)

    shared = {
        "cond_wT": np.ascontiguousarray(cond_w.T),
        "qkv_wT": np.ascontiguousarray(qkv_w.T),
        "proj_wT": np.ascontiguousarray(proj_w.T),
        "w1T": np.ascontiguousarray(mlp_w1.T),
        "w2T": np.ascontiguousarray(mlp_w2.T),
    }
    if flags[0]:
        shared["cond_b"] = cond_b
    if flags[1]:
        shared["qkv_b"] = qkv_b
    if flags[2]:
        shared["proj_b"] = proj_b
    if flags[3]:
        shared["mlp_b1"] = mlp_b1
    if flags[4]:
        shared["mlp_b2"] = mlp_b2

    in_maps = [dict(shared, xb=np.ascontiguousarray(x[b]), cond=np.ascontiguousarray(cond[b]))
               for b in range(B)]
    res = run_bass_kernel_spmd(nc, in_maps, list(range(B)))
    return np.stack([res.results[b]["out"] for b in range(B)], axis=0)
